# revision 15
# baseline (speedup 1.0000x reference)
"""Trainium2 Bass kernel for quantized (Q4_0) multi-head attention prefill.

Problem: nn_Attention_32023276159509
  B=1, S=2048, DIM=4096, 32 q-heads / 8 kv-heads (GQA x4), head_dim=128,
  Q4_0-packed int4 weights with per-64-group fp32 scales, RoPE (rotate-half),
  causal mask, softmax, output projection.

Sharding: tensor-parallel over heads across 8 NeuronCores. Core c owns
q-heads [4c, 4c+4), kv-head c, and wo input-columns [512c, 512(c+1)).
Each core computes a full [S, DIM] partial output; partials are summed on
the host (the all-reduce of the reference sharding recipe).

v2 design (vs baseline):
  - Weights are dequantized on the HOST (exact fp32 math identical to the
    reference) and shipped as W.T planes: the entire on-device dequant
    pipeline (int widen / nibble shifts / scale broadcast matmuls) is gone,
    so phase 1 is a pure DMA->matmul stream.
  - Phase 3 (output projection) is fused into the attention phase per
    q-block, filling PE gaps left by the softmax dependency chain.
  - bf16 for the softmax-linear operands (E, V-natural, attention output,
    Wo) -- same PE rate as fp32r at these shapes, half the SBUF/DMA.
  - Scores path (x, Wqkv, Q, K) stays fp32 exact.
  - The causal mask's diagonal 512x512 blocks are identical; one block is
    shipped and loaded once.
"""
import sys
import numpy as np

sys.path.insert(0, "/opt/trn_rl_repo")

import concourse.bass as bass  # noqa: E402
import concourse.tile as tile  # noqa: E402
from concourse import bacc, mybir, bass_utils  # noqa: E402
from contextlib import ExitStack  # noqa: E402
import ml_dtypes  # noqa: E402

F32 = mybir.dt.float32
F32R = mybir.dt.float32r
BF16 = mybir.dt.bfloat16
I32 = mybir.dt.int32
AOT = mybir.AluOpType
AFT = mybir.ActivationFunctionType

GROUP = 64
DIM = 4096
N_HEADS = 32
N_KV = 8
HEAD_DIM = 128
S = 2048
NCORES = 8
H_LOC = N_HEADS // NCORES          # 4 local q heads
QDIM_LOC = H_LOC * HEAD_DIM        # 512
SCALE = 1.0 / np.sqrt(np.float32(HEAD_DIM))
NEG = -1e9

QB = 512                            # q-block (seq columns per attention tile)
NQB = S // QB                       # 4
NKB = S // 128                      # 16 k-tiles of 128


def _build_kernel(causal: bool):
    """Build + compile the per-core Bass module. Same program on all cores."""
    nc = bacc.Bacc("TRN2", target_bir_lowering=False, debug=False)

    # ---- DRAM tensors (per-core inputs) ----
    xT_d = nc.dram_tensor("xT", [DIM, S], F32R, kind="ExternalInput")
    wqkvT_d = nc.dram_tensor("wqkvT", [DIM, 768], F32R, kind="ExternalInput")
    woT_d = nc.dram_tensor("woT", [QDIM_LOC, DIM], BF16, kind="ExternalInput")
    cosT_d = nc.dram_tensor("cosT", [128, S], F32, kind="ExternalInput")
    sinT_d = nc.dram_tensor("sinT", [128, S], F32, kind="ExternalInput")
    if causal:
        mask_d = nc.dram_tensor("maskb", [QB, QB], F32, kind="ExternalInput")
    else:
        mask_d = nc.dram_tensor("maskT", [S, S], F32, kind="ExternalInput")
    out_d = nc.dram_tensor("out_partial", [S, DIM], F32, kind="ExternalOutput")

    with tile.TileContext(nc) as tc:
        with ExitStack() as top:
            # ---- persistent small constants ----
            cpool = top.enter_context(tc.tile_pool(name="const", bufs=1))
            ones_col = cpool.tile([128, 1], BF16, tag="ones_col")
            nc.vector.memset(ones_col[:], 1.0)
            iden_i = cpool.tile([128, 128], I32, tag="iden_i")
            nc.gpsimd.iota(iden_i[:], pattern=[[1, 128]], base=0, channel_multiplier=-1)
            ident = cpool.tile([128, 128], F32R, tag="ident")
            nc.vector.tensor_scalar(ident[:], iden_i[:], 0, None, AOT.is_equal)

            # ---- persistent activations ----
            qkv_pool = top.enter_context(tc.tile_pool(name="qkv", bufs=1))
            vt_pool = top.enter_context(tc.tile_pool(name="vt", bufs=1))
            QT = [qkv_pool.tile([128, S], F32R, tag=f"qt{h}", name=f"qt{h}")
                  for h in range(H_LOC)]
            KT = qkv_pool.tile([128, S], F32R, tag="kt")
            Vn = qkv_pool.tile([128, S], BF16, tag="vn")   # V natural: [k % 128, (kb, hd)]

            # =================== Phase 1: QKV projections + RoPE ===================
            with ExitStack() as p1:
                w_pool = p1.enter_context(tc.tile_pool(name="wqkv", bufs=1))
                trig = p1.enter_context(tc.tile_pool(name="trig", bufs=1))
                xt_pool = p1.enter_context(tc.tile_pool(name="xt", bufs=6))
                rope_t = p1.enter_context(tc.tile_pool(name="rope", bufs=1))
                ps1 = p1.enter_context(tc.tile_pool(name="ps1", bufs=1, space="PSUM"))

                # all 32 W.T tiles stream in up front (12.6 MB); each tile is
                # split into 3 descriptor chains so multiple DMA engines work
                # on it concurrently (one dma_start = one engine ~22.5 GB/s)
                WQKV = []
                for J in range(32):
                    wt = w_pool.tile([128, 768], F32R, tag=f"w{J}", name=f"w{J}")
                    for c in range(3):
                        nc.sync.dma_start(
                            wt[:, c * 256:(c + 1) * 256],
                            wqkvT_d.ap()[J * 128:(J + 1) * 128,
                                         c * 256:(c + 1) * 256])
                    WQKV.append(wt)

                # trig tiles; sign of rotate-half folded into sinTs rows [0:64)
                cosT = trig.tile([128, S], F32, tag="cosT")
                nc.sync.dma_start(cosT[:], cosT_d.ap())
                sinTs = trig.tile([128, S], F32, tag="sinTs")
                nc.sync.dma_start(sinTs[:], sinT_d.ap())
                nc.vector.tensor_scalar(sinTs[0:64, :], sinTs[0:64, :], -1.0, None,
                                        AOT.mult)

                VT = vt_pool.tile([128, S], F32R, tag="vtt")

                for qb in range(NQB):
                    sl = slice(qb * QB, (qb + 1) * QB)
                    psQ = [ps1.tile([128, QB], F32, tag=f"psq{h}", name=f"psq{h}")
                           for h in range(H_LOC)]
                    psK = ps1.tile([128, QB], F32, tag="psk")
                    psV = ps1.tile([128, QB], F32, tag="psv")
                    for J in range(32):
                        xt = xt_pool.tile([128, QB], F32R, tag="xt")
                        # two chains per tile -> two DMA engines per tile
                        nc.scalar.dma_start(xt[0:64, :],
                                            xT_d.ap()[J * 128:J * 128 + 64, sl])
                        nc.scalar.dma_start(xt[64:128, :],
                                            xT_d.ap()[J * 128 + 64:(J + 1) * 128, sl])
                        st, sp = (J == 0), (J == 31)
                        for h in range(H_LOC):
                            nc.tensor.matmul(psQ[h][:],
                                             WQKV[J][:, h * 128:(h + 1) * 128],
                                             xt[:], start=st, stop=sp)
                        nc.tensor.matmul(psK[:], WQKV[J][:, 512:640], xt[:],
                                         start=st, stop=sp)
                        nc.tensor.matmul(psV[:], WQKV[J][:, 640:768], xt[:],
                                         start=st, stop=sp)

                    # Evacuate all PSUM accumulators first (frees banks for
                    # the next qb's matmuls) on the ACT engine, so the DVE
                    # rope math below never blocks the PE's next qb.
                    raws = []
                    for h in range(H_LOC):
                        raw = rope_t.tile([128, QB], F32, tag=f"raw{h}",
                                          name=f"raw{h}", bufs=2)
                        nc.scalar.activation(raw[:], psQ[h][:], AFT.Copy)
                        raws.append(raw)
                    rawk = rope_t.tile([128, QB], F32, tag="rawk", name="rawk",
                                       bufs=1)
                    nc.scalar.activation(rawk[:], psK[:], AFT.Copy)
                    nc.scalar.activation(VT[:, sl], psV[:], AFT.Copy)

                    def rope_finish(raw, dst):
                        rot = rope_t.tile([128, QB], F32, tag="rot", name="rot",
                                          bufs=2)
                        nc.sync.dma_start(rot[0:64, :], raw[64:128, :])
                        nc.sync.dma_start(rot[64:128, :], raw[0:64, :])
                        t1 = rope_t.tile([128, QB], F32, tag="t1", name="t1",
                                         bufs=2)
                        nc.vector.tensor_tensor(t1[:], raw[:], cosT[:, sl], AOT.mult)
                        nc.vector.tensor_tensor(rot[:], rot[:], sinTs[:, sl],
                                                AOT.mult)
                        nc.vector.tensor_tensor(dst[:, sl], t1[:], rot[:], AOT.add)

                    for h in range(H_LOC):
                        rope_finish(raws[h], QT[h])
                    rope_finish(rawk, KT)



            # =================== Phase 2: attention + output projection ===========
            aot_pool = top.enter_context(tc.tile_pool(name="aotp", bufs=1))
            AOT_t = [aot_pool.tile([128, S], BF16, tag=f"aot{h}", name=f"aot{h}")
                     for h in range(H_LOC)]
            wo_pool = top.enter_context(tc.tile_pool(name="wo", bufs=1))

            with ExitStack() as p2:
                mk_pool = p2.enter_context(tc.tile_pool(name="mk", bufs=1))
                mkg_pool = p2.enter_context(tc.tile_pool(name="mkg", bufs=4))
                e_pool = p2.enter_context(tc.tile_pool(name="ep", bufs=4))
                at_pool = p2.enter_context(tc.tile_pool(name="at", bufs=2))
                stage_pool = p2.enter_context(tc.tile_pool(name="stage", bufs=4))
                ps_s = p2.enter_context(tc.tile_pool(name="ps_s", bufs=3, space="PSUM"))
                ps_d = p2.enter_context(tc.tile_pool(name="ps_d", bufs=1, space="PSUM"))
                ps_av = p2.enter_context(tc.tile_pool(name="ps_av", bufs=2, space="PSUM"))
                ps_o = p2.enter_context(tc.tile_pool(name="ps_o", bufs=2, space="PSUM"))

                # causal: the 4 diagonal-block mask tiles (shared by all qb).
                # Issued before the Wo stream so qb0's attention isn't stuck
                # behind 4MB of weight DMA.
                mtiles = []
                if causal:
                    for i in range(4):
                        mt = mk_pool.tile([128, QB], F32, tag=f"mk{i}", name=f"mk{i}")
                        nc.sync.dma_start(
                            mt[:], mask_d.ap()[i * 128:(i + 1) * 128, :])
                        mtiles.append(mt)

                # Wo (bf16, host-dequantized) resident: 4 head-tiles [128, 4096]
                WOt = []
                for J in range(H_LOC):
                    wot = wo_pool.tile([128, DIM], BF16, tag=f"wo{J}", name=f"wo{J}")
                    nc.sync.dma_start(wot[:], woT_d.ap()[J * 128:(J + 1) * 128, :])
                    WOt.append(wot)

                def proj_chunk(sb):
                    """Output projection for one 128-row seq block."""
                    ssl = slice(sb * 128, (sb + 1) * 128)
                    for ob in range(DIM // 512):
                        osl = slice(ob * 512, (ob + 1) * 512)
                        psO = ps_o.tile([128, 512], F32, tag="pso", name="pso")
                        for J in range(H_LOC):
                            nc.tensor.matmul(psO[:], AOT_t[J][:, ssl],
                                             WOt[J][:, osl],
                                             start=(J == 0), stop=(J == 3))
                        ot = stage_pool.tile([128, 512], F32, tag="ot",
                                             name="ot")
                        nc.vector.tensor_copy(ot[:], psO[:])
                        nc.sync.dma_start(out_d.ap()[ssl, osl], ot[:])

                for qb in range(NQB):
                    sl = slice(qb * QB, (qb + 1) * QB)
                    nkb = 4 * (qb + 1) if causal else NKB
                    # V.T -> V natural (bf16) for this qb's new k-tiles
                    for kb in range(4 * qb, 4 * qb + 4):
                        pv = ps_s.tile([128, 128], F32R, tag="pss", name="pvt")
                        nc.tensor.transpose(pv[:], VT[:, kb * 128:(kb + 1) * 128],
                                            ident[:])
                        nc.vector.tensor_copy(Vn[:, kb * 128:(kb + 1) * 128], pv[:])
                    for h in range(H_LOC):
                        psD = ps_d.tile([1, QB], F32, tag="psd")
                        psAV = ps_av.tile([128, QB], F32, tag="psav")

                        def exp_tile(kb):
                            psS = ps_s.tile([128, QB], F32, tag="pss", name="pss")
                            nc.tensor.matmul(psS[:],
                                             KT[:, kb * 128:(kb + 1) * 128],
                                             QT[h][:, sl], start=True, stop=True)
                            E = e_pool.tile([128, QB], BF16, tag="e", name="e")
                            if causal and kb >= 4 * qb:
                                mt = mtiles[kb - 4 * qb]
                            elif not causal:
                                mt = mkg_pool.tile([128, QB], F32,
                                                   tag=f"mkg{kb % 4}",
                                                   name=f"mkg{kb}")
                                nc.sync.dma_start(
                                    mt[:],
                                    mask_d.ap()[kb * 128:(kb + 1) * 128, sl])
                            else:
                                mt = None
                            if mt is not None:
                                tmp = at_pool.tile([128, QB], F32, tag="sm",
                                                   name="sm")
                                nc.vector.scalar_tensor_tensor(
                                    tmp[:], psS[:], float(SCALE), mt[:],
                                    AOT.mult, AOT.add)
                                nc.scalar.activation(E[:], tmp[:], AFT.Exp)
                            else:
                                nc.scalar.activation(E[:], psS[:], AFT.Exp,
                                                     scale=float(SCALE))
                            return E

                        # software pipeline, depth 2: score/exp runs two k-tiles
                        # ahead of the denominator/AV matmuls, hiding the
                        # psS -> (mask-add) -> exp latency chain.
                        Eq = [exp_tile(0)]
                        if nkb > 1:
                            Eq.append(exp_tile(1))
                        for kb in range(nkb):
                            if kb + 2 < nkb:
                                Eq.append(exp_tile(kb + 2))
                            st, sp = (kb == 0), (kb == nkb - 1)
                            Ecur = Eq.pop(0)
                            nc.tensor.matmul(psD[:], ones_col[:], Ecur[:],
                                             start=st, stop=sp)
                            nc.tensor.matmul(psAV[:],
                                             Vn[:, kb * 128:(kb + 1) * 128],
                                             Ecur[:], start=st, stop=sp)
                        dsb = at_pool.tile([1, QB], F32, tag="dsb", name="dsb")
                        nc.scalar.activation(dsb[:], psD[:], AFT.Copy)
                        # interleave the previous q-block's output projection
                        # between attention heads to fill softmax-chain stalls.
                        # Emitted BEFORE the reciprocal so the psO evacuations
                        # sit ahead of it in the in-order DVE queue.
                        if qb > 0:
                            proj_chunk(4 * (qb - 1) + h)
                        rec = at_pool.tile([1, QB], F32, tag="rec", name="rec")
                        nc.vector.reciprocal(rec[:], dsb[:])
                        db = at_pool.tile([128, QB], F32, tag="db")
                        nc.gpsimd.partition_broadcast(db[:], rec[:])
                        nc.vector.tensor_tensor(AOT_t[h][:, sl], psAV[:], db[:],
                                                AOT.mult)
                    if qb == NQB - 1:
                        for i in range(4):
                            proj_chunk(4 * qb + i)

    nc.compile()
    return nc


_BUILD_CACHE = {}


def _get_kernel(causal: bool):
    if causal not in _BUILD_CACHE:
        _BUILD_CACHE[causal] = _build_kernel(causal)
    return _BUILD_CACHE[causal]


def _dequant_np(packed, scales, out_f, in_f):
    """Exact numpy port of the reference dequantize_q40."""
    w = np.asarray(packed)
    if w.dtype != np.int8:
        w = w.astype(np.int8)
    s = np.asarray(scales, dtype=np.float32).reshape(-1, 1)
    msb = w >> 4                                   # arithmetic, sign-extends
    lsb = (w << 4) >> 4                            # int8 wraps, then sign-extends
    grp = np.concatenate([msb, lsb], axis=1).reshape(-1, GROUP).astype(np.float32)
    return (grp * s).reshape(out_f, in_f)


def _canonical_causal_mask():
    causal = np.triu(np.ones((S, S), dtype=bool), k=1)
    return np.where(causal, np.float32(NEG), np.float32(0.0)).astype(np.float32)


def build_in_maps(inputs, causal):
    x = np.asarray(inputs["x"], dtype=np.float32)
    cos = np.asarray(inputs["cos"], dtype=np.float32)
    sin = np.asarray(inputs["sin"], dtype=np.float32)
    mask = np.asarray(inputs["mask"], dtype=np.float32)

    xT = np.ascontiguousarray(x.reshape(S, DIM).T)                 # [DIM, S]
    cosT = np.ascontiguousarray(np.concatenate([cos.T, cos.T], axis=0))  # [128, S]
    sinT = np.ascontiguousarray(np.concatenate([sin.T, sin.T], axis=0))

    Wq = _dequant_np(inputs["wq"], inputs["sq"], N_HEADS * HEAD_DIM, DIM)
    Wk = _dequant_np(inputs["wk"], inputs["sk"], N_KV * HEAD_DIM, DIM)
    Wv = _dequant_np(inputs["wv"], inputs["sv"], N_KV * HEAD_DIM, DIM)
    Wo = _dequant_np(inputs["wo"], inputs["so"], DIM, N_HEADS * HEAD_DIM)

    in_maps = []
    for c in range(NCORES):
        q0 = c * QDIM_LOC
        k0 = c * HEAD_DIM
        wqkvT = np.empty((DIM, 768), dtype=np.float32)
        wqkvT[:, 0:512] = Wq[q0:q0 + QDIM_LOC].T
        wqkvT[:, 512:640] = Wk[k0:k0 + HEAD_DIM].T
        wqkvT[:, 640:768] = Wv[k0:k0 + HEAD_DIM].T
        woT = np.ascontiguousarray(
            Wo[:, q0:q0 + QDIM_LOC].T).astype(ml_dtypes.bfloat16)  # [512, 4096]
        m = dict(xT=xT, wqkvT=wqkvT, woT=woT, cosT=cosT, sinT=sinT)
        if causal:
            m["maskb"] = np.ascontiguousarray(mask[0:QB, 0:QB].T)
        else:
            m["maskT"] = np.ascontiguousarray(mask.T)
        in_maps.append(m)
    return in_maps


def kernel(**inputs):
    mask = np.asarray(inputs["mask"], dtype=np.float32)
    causal = bool(np.array_equal(mask, _canonical_causal_mask()))
    nc = _get_kernel(causal)
    in_maps = build_in_maps(inputs, causal)
    res = bass_utils.run_bass_kernel_spmd(nc, in_maps, core_ids=list(range(NCORES)))
    acc = np.zeros((S, DIM), dtype=np.float64)
    for r in res.results:
        acc += r["out_partial"].astype(np.float64)
    return acc.astype(np.float32).reshape(1, S, DIM)


if __name__ == "__main__":
    print("building causal kernel...")
    _get_kernel(True)
    print("built")


# revision 18
# speedup vs baseline: 1.0352x; 1.0352x over previous
"""Trainium2 Bass kernel for quantized (Q4_0) multi-head attention prefill.

Problem: nn_Attention_32023276159509
  B=1, S=2048, DIM=4096, 32 q-heads / 8 kv-heads (GQA x4), head_dim=128,
  Q4_0-packed int4 weights with per-64-group fp32 scales, RoPE (rotate-half),
  causal mask, softmax, output projection.

Sharding: tensor-parallel over heads across 8 NeuronCores. Core c owns
q-heads [4c, 4c+4), kv-head c, and wo input-columns [512c, 512(c+1)).
Each core computes a full [S, DIM] partial output; partials are summed on
the host (the all-reduce of the reference sharding recipe).

v2 design (vs baseline):
  - Weights are dequantized on the HOST (exact fp32 math identical to the
    reference) and shipped as W.T planes: the entire on-device dequant
    pipeline (int widen / nibble shifts / scale broadcast matmuls) is gone,
    so phase 1 is a pure DMA->matmul stream.
  - Phase 3 (output projection) is fused into the attention phase per
    q-block, filling PE gaps left by the softmax dependency chain.
  - bf16 for the softmax-linear operands (E, V-natural, attention output,
    Wo) -- same PE rate as fp32r at these shapes, half the SBUF/DMA.
  - Scores path (x, Wqkv, Q, K) stays fp32 exact.
  - The causal mask's diagonal 512x512 blocks are identical; one block is
    shipped and loaded once.
"""
import sys
import numpy as np

sys.path.insert(0, "/opt/trn_rl_repo")

import concourse.bass as bass  # noqa: E402
import concourse.tile as tile  # noqa: E402
from concourse import bacc, mybir, bass_utils  # noqa: E402
from contextlib import ExitStack  # noqa: E402
import ml_dtypes  # noqa: E402

F32 = mybir.dt.float32
F32R = mybir.dt.float32r
BF16 = mybir.dt.bfloat16
I32 = mybir.dt.int32
AOT = mybir.AluOpType
AFT = mybir.ActivationFunctionType

GROUP = 64
DIM = 4096
N_HEADS = 32
N_KV = 8
HEAD_DIM = 128
S = 2048
NCORES = 8
H_LOC = N_HEADS // NCORES          # 4 local q heads
QDIM_LOC = H_LOC * HEAD_DIM        # 512
SCALE = 1.0 / np.sqrt(np.float32(HEAD_DIM))
NEG = -1e9

QB = 512                            # q-block (seq columns per attention tile)
NQB = S // QB                       # 4
NKB = S // 128                      # 16 k-tiles of 128


def _build_kernel(causal: bool):
    """Build + compile the per-core Bass module. Same program on all cores."""
    nc = bacc.Bacc("TRN2", target_bir_lowering=False, debug=False)

    # ---- DRAM tensors (per-core inputs) ----
    xT_d = nc.dram_tensor("xT", [DIM, S], F32R, kind="ExternalInput")
    wqkvT_d = nc.dram_tensor("wqkvT", [DIM, 768], F32R, kind="ExternalInput")
    woT_d = nc.dram_tensor("woT", [QDIM_LOC, DIM], BF16, kind="ExternalInput")
    cosT_d = nc.dram_tensor("cosT", [128, S], F32, kind="ExternalInput")
    sinT_d = nc.dram_tensor("sinT", [128, S], F32, kind="ExternalInput")
    if causal:
        mask_d = nc.dram_tensor("maskb", [QB, QB], F32, kind="ExternalInput")
    else:
        mask_d = nc.dram_tensor("maskT", [S, S], F32, kind="ExternalInput")
    out_d = nc.dram_tensor("out_partial", [S, DIM], F32, kind="ExternalOutput")

    with tile.TileContext(nc) as tc:
        with ExitStack() as top:
            # ---- persistent small constants ----
            cpool = top.enter_context(tc.tile_pool(name="const", bufs=1))
            ones_col = cpool.tile([128, 1], BF16, tag="ones_col")
            nc.vector.memset(ones_col[:], 1.0)
            iden_i = cpool.tile([128, 128], I32, tag="iden_i")
            nc.gpsimd.iota(iden_i[:], pattern=[[1, 128]], base=0, channel_multiplier=-1)
            ident = cpool.tile([128, 128], F32R, tag="ident")
            nc.vector.tensor_scalar(ident[:], iden_i[:], 0, None, AOT.is_equal)

            # ---- persistent activations ----
            qkv_pool = top.enter_context(tc.tile_pool(name="qkv", bufs=1))
            vt_pool = top.enter_context(tc.tile_pool(name="vt", bufs=1))
            QT = [qkv_pool.tile([128, S], F32R, tag=f"qt{h}", name=f"qt{h}")
                  for h in range(H_LOC)]
            KT = qkv_pool.tile([128, S], F32R, tag="kt")
            Vn = qkv_pool.tile([128, S], BF16, tag="vn")   # V natural: [k % 128, (kb, hd)]

            # =================== Phase 1: QKV projections + RoPE ===================
            with ExitStack() as p1:
                w_pool = p1.enter_context(tc.tile_pool(name="wqkv", bufs=1))
                trig = p1.enter_context(tc.tile_pool(name="trig", bufs=1))
                xt_pool = p1.enter_context(tc.tile_pool(name="xt", bufs=6))
                rope_t = p1.enter_context(tc.tile_pool(name="rope", bufs=1))
                ps1 = p1.enter_context(tc.tile_pool(name="ps1", bufs=1, space="PSUM"))

                # all 32 W.T tiles stream in up front (12.6 MB); each tile is
                # split into 3 descriptor chains so multiple DMA engines work
                # on it concurrently (one dma_start = one engine ~22.5 GB/s)
                WQKV = []
                for J in range(32):
                    wt = w_pool.tile([128, 768], F32R, tag=f"w{J}", name=f"w{J}")
                    for c in range(3):
                        nc.sync.dma_start(
                            wt[:, c * 256:(c + 1) * 256],
                            wqkvT_d.ap()[J * 128:(J + 1) * 128,
                                         c * 256:(c + 1) * 256])
                    WQKV.append(wt)

                # trig tiles; sign of rotate-half folded into sinTs rows [0:64)
                cosT = trig.tile([128, S], F32, tag="cosT")
                nc.sync.dma_start(cosT[:], cosT_d.ap())
                sinTs = trig.tile([128, S], F32, tag="sinTs")
                nc.sync.dma_start(sinTs[:], sinT_d.ap())
                nc.vector.tensor_scalar(sinTs[0:64, :], sinTs[0:64, :], -1.0, None,
                                        AOT.mult)

                VT = vt_pool.tile([128, S], F32R, tag="vtt")

                for qb in range(NQB):
                    sl = slice(qb * QB, (qb + 1) * QB)
                    psQ = [ps1.tile([128, QB], F32, tag=f"psq{h}", name=f"psq{h}")
                           for h in range(H_LOC)]
                    psK = ps1.tile([128, QB], F32, tag="psk")
                    psV = ps1.tile([128, QB], F32, tag="psv")
                    for J in range(32):
                        xt = xt_pool.tile([128, QB], F32R, tag="xt")
                        # two chains per tile -> two DMA engines per tile
                        nc.scalar.dma_start(xt[0:64, :],
                                            xT_d.ap()[J * 128:J * 128 + 64, sl])
                        nc.scalar.dma_start(xt[64:128, :],
                                            xT_d.ap()[J * 128 + 64:(J + 1) * 128, sl])
                        st, sp = (J == 0), (J == 31)
                        for h in range(H_LOC):
                            nc.tensor.matmul(psQ[h][:],
                                             WQKV[J][:, h * 128:(h + 1) * 128],
                                             xt[:], start=st, stop=sp)
                        nc.tensor.matmul(psK[:], WQKV[J][:, 512:640], xt[:],
                                         start=st, stop=sp)
                        nc.tensor.matmul(psV[:], WQKV[J][:, 640:768], xt[:],
                                         start=st, stop=sp)

                    # Evacuate all PSUM accumulators first (frees banks for
                    # the next qb's matmuls) on the ACT engine, so the DVE
                    # rope math below never blocks the PE's next qb.
                    raws = []
                    for h in range(H_LOC):
                        raw = rope_t.tile([128, QB], F32, tag=f"raw{h}",
                                          name=f"raw{h}", bufs=2)
                        nc.scalar.activation(raw[:], psQ[h][:], AFT.Copy)
                        raws.append(raw)
                    rawk = rope_t.tile([128, QB], F32, tag="rawk", name="rawk",
                                       bufs=1)
                    nc.scalar.activation(rawk[:], psK[:], AFT.Copy)
                    nc.scalar.activation(VT[:, sl], psV[:], AFT.Copy)

                    def rope_finish(raw, dst):
                        rot = rope_t.tile([128, QB], F32, tag="rot", name="rot",
                                          bufs=2)
                        # gpsimd issue queue: idle in phase 1, 25ns issue cost,
                        # and crucially NOT behind the 96 W-chunk issues on sync
                        nc.gpsimd.dma_start(rot[0:64, :], raw[64:128, :])
                        nc.gpsimd.dma_start(rot[64:128, :], raw[0:64, :])
                        t1 = rope_t.tile([128, QB], F32, tag="t1", name="t1",
                                         bufs=2)
                        nc.vector.tensor_tensor(t1[:], raw[:], cosT[:, sl], AOT.mult)
                        nc.vector.tensor_tensor(rot[:], rot[:], sinTs[:, sl],
                                                AOT.mult)
                        nc.vector.tensor_tensor(dst[:, sl], t1[:], rot[:], AOT.add)

                    for h in range(H_LOC):
                        rope_finish(raws[h], QT[h])
                    rope_finish(rawk, KT)



            # =================== Phase 2: attention + output projection ===========
            aot_pool = top.enter_context(tc.tile_pool(name="aotp", bufs=1))
            AOT_t = [aot_pool.tile([128, S], BF16, tag=f"aot{h}", name=f"aot{h}")
                     for h in range(H_LOC)]
            wo_pool = top.enter_context(tc.tile_pool(name="wo", bufs=1))

            with ExitStack() as p2:
                mk_pool = p2.enter_context(tc.tile_pool(name="mk", bufs=1))
                mkg_pool = p2.enter_context(tc.tile_pool(name="mkg", bufs=4))
                e_pool = p2.enter_context(tc.tile_pool(name="ep", bufs=5))
                at_pool = p2.enter_context(tc.tile_pool(name="at", bufs=2))
                stage_pool = p2.enter_context(tc.tile_pool(name="stage", bufs=4))
                ps_s = p2.enter_context(tc.tile_pool(name="ps_s", bufs=3, space="PSUM"))
                ps_d = p2.enter_context(tc.tile_pool(name="ps_d", bufs=1, space="PSUM"))
                ps_av = p2.enter_context(tc.tile_pool(name="ps_av", bufs=2, space="PSUM"))
                ps_o = p2.enter_context(tc.tile_pool(name="ps_o", bufs=2, space="PSUM"))

                # causal: the 4 diagonal-block mask tiles (shared by all qb).
                # Issued before the Wo stream so qb0's attention isn't stuck
                # behind 4MB of weight DMA.
                mtiles = []
                if causal:
                    for i in range(4):
                        mt = mk_pool.tile([128, QB], F32, tag=f"mk{i}", name=f"mk{i}")
                        nc.sync.dma_start(
                            mt[:], mask_d.ap()[i * 128:(i + 1) * 128, :])
                        mtiles.append(mt)

                # Wo (bf16, host-dequantized) resident: 4 head-tiles [128, 4096]
                WOt = []
                for J in range(H_LOC):
                    wot = wo_pool.tile([128, DIM], BF16, tag=f"wo{J}", name=f"wo{J}")
                    nc.sync.dma_start(wot[:], woT_d.ap()[J * 128:(J + 1) * 128, :])
                    WOt.append(wot)

                def proj_chunk(sb):
                    """Output projection for one 128-row seq block."""
                    ssl = slice(sb * 128, (sb + 1) * 128)
                    for ob in range(DIM // 512):
                        osl = slice(ob * 512, (ob + 1) * 512)
                        psO = ps_o.tile([128, 512], F32, tag="pso", name="pso")
                        for J in range(H_LOC):
                            nc.tensor.matmul(psO[:], AOT_t[J][:, ssl],
                                             WOt[J][:, osl],
                                             start=(J == 0), stop=(J == 3))
                        ot = stage_pool.tile([128, 512], F32, tag="ot",
                                             name="ot")
                        nc.vector.tensor_copy(ot[:], psO[:])
                        nc.gpsimd.dma_start(out_d.ap()[ssl, osl], ot[:])

                def exp_tile(qb, h, kb):
                    sl = slice(qb * QB, (qb + 1) * QB)
                    psS = ps_s.tile([128, QB], F32, tag="pss", name="pss")
                    nc.tensor.matmul(psS[:],
                                     KT[:, kb * 128:(kb + 1) * 128],
                                     QT[h][:, sl], start=True, stop=True)
                    E = e_pool.tile([128, QB], BF16, tag="e", name="e")
                    if causal and kb >= 4 * qb:
                        mt = mtiles[kb - 4 * qb]
                    elif not causal:
                        mt = mkg_pool.tile([128, QB], F32, tag=f"mkg{kb % 4}",
                                           name=f"mkg{kb}")
                        nc.sync.dma_start(
                            mt[:], mask_d.ap()[kb * 128:(kb + 1) * 128, sl])
                    else:
                        mt = None
                    if mt is not None:
                        tmp = at_pool.tile([128, QB], F32, tag="sm", name="sm")
                        nc.vector.scalar_tensor_tensor(
                            tmp[:], psS[:], float(SCALE), mt[:],
                            AOT.mult, AOT.add)
                        nc.scalar.activation(E[:], tmp[:], AFT.Exp)
                    else:
                        nc.scalar.activation(E[:], psS[:], AFT.Exp,
                                             scale=float(SCALE))
                    return E

                # Flat schedule over (qb, h, kb) with a global score/exp
                # pipeline LOOKAHEAD tiles deep -- the E queue stays primed
                # across head and q-block boundaries, so the PE's D/AV
                # matmuls never wait on the psS -> mask-add -> exp chain.
                flat = []
                for qb in range(NQB):
                    nkb = 4 * (qb + 1) if causal else NKB
                    for h in range(H_LOC):
                        for kb in range(nkb):
                            flat.append((qb, h, kb, nkb))
                LOOKAHEAD = 3
                Equeue = []
                pidx = 0
                for idx, (qb, h, kb, nkb) in enumerate(flat):
                    if h == 0 and kb == 0:
                        # V.T -> V natural (bf16) for this qb's new k-tiles
                        for tkb in range(4 * qb, 4 * qb + 4):
                            pv = ps_s.tile([128, 128], F32R, tag="pss",
                                           name="pvt")
                            nc.tensor.transpose(
                                pv[:], VT[:, tkb * 128:(tkb + 1) * 128],
                                ident[:])
                            nc.vector.tensor_copy(
                                Vn[:, tkb * 128:(tkb + 1) * 128], pv[:])
                    while pidx < len(flat) and pidx <= idx + LOOKAHEAD:
                        pq, ph, pk, _ = flat[pidx]
                        Equeue.append(exp_tile(pq, ph, pk))
                        pidx += 1
                    if kb == 0:
                        psD = ps_d.tile([1, QB], F32, tag="psd")
                        psAV = ps_av.tile([128, QB], F32, tag="psav")
                    st, sp = (kb == 0), (kb == nkb - 1)
                    Ecur = Equeue.pop(0)
                    nc.tensor.matmul(psD[:], ones_col[:], Ecur[:],
                                     start=st, stop=sp)
                    nc.tensor.matmul(psAV[:],
                                     Vn[:, kb * 128:(kb + 1) * 128],
                                     Ecur[:], start=st, stop=sp)
                    if sp:
                        sl = slice(qb * QB, (qb + 1) * QB)
                        dsb = at_pool.tile([1, QB], F32, tag="dsb", name="dsb")
                        nc.scalar.activation(dsb[:], psD[:], AFT.Copy)
                        # interleave the previous q-block's output projection:
                        # fills the softmax normalization latency, and its psO
                        # evacuations sit ahead of the reciprocal in the
                        # in-order DVE queue.
                        if qb > 0:
                            proj_chunk(4 * (qb - 1) + h)
                        rec = at_pool.tile([1, QB], F32, tag="rec", name="rec")
                        nc.vector.reciprocal(rec[:], dsb[:])
                        db = at_pool.tile([128, QB], F32, tag="db")
                        nc.gpsimd.partition_broadcast(db[:], rec[:])
                        nc.vector.tensor_tensor(AOT_t[h][:, sl], psAV[:],
                                                db[:], AOT.mult)
                for i in range(4):
                    proj_chunk(4 * (NQB - 1) + i)

    nc.compile()
    return nc


_BUILD_CACHE = {}


def _get_kernel(causal: bool):
    if causal not in _BUILD_CACHE:
        _BUILD_CACHE[causal] = _build_kernel(causal)
    return _BUILD_CACHE[causal]


def _dequant_np(packed, scales, out_f, in_f):
    """Exact numpy port of the reference dequantize_q40."""
    w = np.asarray(packed)
    if w.dtype != np.int8:
        w = w.astype(np.int8)
    s = np.asarray(scales, dtype=np.float32).reshape(-1, 1)
    msb = w >> 4                                   # arithmetic, sign-extends
    lsb = (w << 4) >> 4                            # int8 wraps, then sign-extends
    grp = np.concatenate([msb, lsb], axis=1).reshape(-1, GROUP).astype(np.float32)
    return (grp * s).reshape(out_f, in_f)


def _canonical_causal_mask():
    causal = np.triu(np.ones((S, S), dtype=bool), k=1)
    return np.where(causal, np.float32(NEG), np.float32(0.0)).astype(np.float32)


def build_in_maps(inputs, causal):
    x = np.asarray(inputs["x"], dtype=np.float32)
    cos = np.asarray(inputs["cos"], dtype=np.float32)
    sin = np.asarray(inputs["sin"], dtype=np.float32)
    mask = np.asarray(inputs["mask"], dtype=np.float32)

    xT = np.ascontiguousarray(x.reshape(S, DIM).T)                 # [DIM, S]
    cosT = np.ascontiguousarray(np.concatenate([cos.T, cos.T], axis=0))  # [128, S]
    sinT = np.ascontiguousarray(np.concatenate([sin.T, sin.T], axis=0))

    Wq = _dequant_np(inputs["wq"], inputs["sq"], N_HEADS * HEAD_DIM, DIM)
    Wk = _dequant_np(inputs["wk"], inputs["sk"], N_KV * HEAD_DIM, DIM)
    Wv = _dequant_np(inputs["wv"], inputs["sv"], N_KV * HEAD_DIM, DIM)
    Wo = _dequant_np(inputs["wo"], inputs["so"], DIM, N_HEADS * HEAD_DIM)

    in_maps = []
    for c in range(NCORES):
        q0 = c * QDIM_LOC
        k0 = c * HEAD_DIM
        wqkvT = np.empty((DIM, 768), dtype=np.float32)
        wqkvT[:, 0:512] = Wq[q0:q0 + QDIM_LOC].T
        wqkvT[:, 512:640] = Wk[k0:k0 + HEAD_DIM].T
        wqkvT[:, 640:768] = Wv[k0:k0 + HEAD_DIM].T
        woT = np.ascontiguousarray(
            Wo[:, q0:q0 + QDIM_LOC].T).astype(ml_dtypes.bfloat16)  # [512, 4096]
        m = dict(xT=xT, wqkvT=wqkvT, woT=woT, cosT=cosT, sinT=sinT)
        if causal:
            m["maskb"] = np.ascontiguousarray(mask[0:QB, 0:QB].T)
        else:
            m["maskT"] = np.ascontiguousarray(mask.T)
        in_maps.append(m)
    return in_maps


def kernel(**inputs):
    mask = np.asarray(inputs["mask"], dtype=np.float32)
    causal = bool(np.array_equal(mask, _canonical_causal_mask()))
    nc = _get_kernel(causal)
    in_maps = build_in_maps(inputs, causal)
    res = bass_utils.run_bass_kernel_spmd(nc, in_maps, core_ids=list(range(NCORES)))
    acc = np.zeros((S, DIM), dtype=np.float64)
    for r in res.results:
        acc += r["out_partial"].astype(np.float64)
    return acc.astype(np.float32).reshape(1, S, DIM)


if __name__ == "__main__":
    print("building causal kernel...")
    _get_kernel(True)
    print("built")
